# revision 34
# baseline (speedup 1.0000x reference)
"""Trainium2 Bass/Tile kernel: symmetric contrastive loss (CLIP-style).

Distribution: data-parallel over B across 8 NeuronCores.  Each core MLPs +
l2-normalizes its 2048-row shard of both branches, AllGathers the normalized
num-projections (bf16, 512KB/rank), computes its row-block of the 16384^2
logit matrix tile-by-tile (never materialized), and reduces:

  * rows  (i2n): ACT Exp with fused accum_out -> per-row sum(exp) locally
  * cols  (n2i): DVE bf16 adds fold the 16 row-chunks of each 2048-wide
    column supertile into an SBUF accumulator; one ones-matmul per supertile
    turns it into colsums; two AllReduce-adds ([8192] mid-loop hidden,
    [8192+scalars] tail) finish the cross-core reduction.

v3: inputs are staged host-side TRANSPOSED (feature-major), so the MLPs
read them directly as matmul operands -- no PE transposes, no PSUM bounce,
no transpose copies (v2 spent ~90us of PE prologue on 288 transposes).
The img MLP1 runs contraction-outer so each arriving k-chunk is consumed
immediately; img chunk DMAs alternate between the SWDGE (cast) and HWDGE
(f32 + DVE cast) queues to use both DMA paths in parallel.

Other structure (from v2): 2048-wide Exp tiles; column sums on DVE;
normalization Ln/Exp batched per branch (ACT stays on the exp table set
through the main loop); dummy 8-element AllGather at t~0 absorbs the
one-time collective bootstrap; colsums DMA'd straight from PSUM.

Logits are bounded (|cos|/temp <= 10) so logsumexp needs no max shift; the
l2 normalization is exp(-0.5*ln(|z|^2) - 0.5*log_temp) on ACT (Rsqrt on ACT
is banned for accuracy), with temperature folded in via the bias.
"""

import numpy as np

N_CORES = 8
B = 16384
D_IMG = 2048
D_NUM = 256
P = 128

_NC_CACHE = {}


def build(b_total=B, d_img=D_IMG, d_num=D_NUM, n_cores=N_CORES):
    """Build + compile the Bass module. Returns the compiled Bacc object."""
    key = (b_total, d_img, d_num, n_cores)
    if key in _NC_CACHE:
        return _NC_CACHE[key]

    try:
        import concourse.bacc as bacc
    except ImportError:
        import sys
        sys.path.insert(0, "/opt/trn_rl_repo")
        import concourse.bacc as bacc
    import concourse.mybir as mybir
    import concourse.tile as tile

    dt = mybir.dt
    AF = mybir.ActivationFunctionType
    Alu = mybir.AluOpType
    AX = mybir.AxisListType
    f32 = dt.float32
    bf16 = dt.bfloat16

    BL = b_total // n_cores          # local rows per core
    assert BL % 512 == 0 and b_total % 2048 == 0
    NRT = BL // 512                  # 512-wide row tiles (MLP)
    NRC = BL // 128                  # 128-row chunks (main pass)
    KI = d_img // 128                # contraction tiles, img MLP1
    KN = d_num // 128
    CW = 2048                        # main-pass column supertile width
    NCT = b_total // CW              # number of supertiles
    NH = CW // 512
    ARW = b_total + 64               # total AllReduce payload width
    ARH = (NCT // 2) * CW            # first AllReduce chunk (cols)

    nc = bacc.Bacc("TRN2", target_bir_lowering=False, debug=False,
                   num_devices=n_cores)

    imgT = nc.dram_tensor("imgT", [d_img, BL], bf16, kind="ExternalInput").ap()
    numT = nc.dram_tensor("numT", [d_num, BL], bf16, kind="ExternalInput").ap()
    Wi1 = nc.dram_tensor("Wi1", [d_img, P], bf16, kind="ExternalInput").ap()
    bi1 = nc.dram_tensor("bi1", [P, 1], f32, kind="ExternalInput").ap()
    Wi2 = nc.dram_tensor("Wi2", [P, P], bf16, kind="ExternalInput").ap()
    bi2 = nc.dram_tensor("bi2", [P, 1], f32, kind="ExternalInput").ap()
    Wn1 = nc.dram_tensor("Wn1", [d_num, P], bf16, kind="ExternalInput").ap()
    bn1 = nc.dram_tensor("bn1", [P, 1], f32, kind="ExternalInput").ap()
    Wn2 = nc.dram_tensor("Wn2", [P, P], bf16, kind="ExternalInput").ap()
    bn2 = nc.dram_tensor("bn2", [P, 1], f32, kind="ExternalInput").ap()
    ltm = nc.dram_tensor("log_temp", [1, 1], f32, kind="ExternalInput").ap()
    loss = nc.dram_tensor("loss", [1, 1], f32, kind="ExternalOutput").ap()

    rg = [list(range(n_cores))]

    with tile.TileContext(nc) as tc:
        with (
            tc.tile_pool(name="sb", bufs=1) as sb,
            tc.tile_pool(name="stream", bufs=3) as st,
            tc.tile_pool(name="vstage", bufs=2) as vs,
            tc.tile_pool(name="dram", bufs=1, space="DRAM") as dram,
        ):
            # ---------------- DRAM scratch ----------------
            dum_in = dram.tile([1, 8], f32)
            dum_out = dram.tile([n_cores, 8], f32, addr_space="Shared")
            SL0 = min(256, BL // 2)   # early AllGather slice width
            ag_in_a = dram.tile([P, SL0], bf16)
            ag_out_a = dram.tile([n_cores * P, SL0], bf16, addr_space="Shared")
            ag_in_b = dram.tile([P, BL - SL0], bf16)
            ag_out_b = dram.tile([n_cores * P, BL - SL0], bf16,
                                 addr_space="Shared")
            ar_in = dram.tile([1, ARW], f32)
            ar_out_a = dram.tile([1, ARH], f32, addr_space="Shared")
            ar_out_b = dram.tile([1, ARW - ARH], f32, addr_space="Shared")

            # ---------------- bootstrap ----------------
            zpad = sb.tile([1, 64], f32)
            nc.vector.memset(zpad[:], 0.0)
            # dummy collective first: absorbs the one-time comm bootstrap
            # (~40us) while the input DMAs and MLPs run.  (The bootstrap's
            # start time is runtime-internal, ~10-22us after NEFF start,
            # regardless of when the trigger's input is ready.)
            nc.sync.dma_start(dum_in[:], zpad[:1, 0:8])
            nc.gpsimd.collective_compute(
                "AllGather", Alu.bypass, replica_groups=rg,
                ins=[dum_in.opt()], outs=[dum_out.opt()])

            # num input: SWDGE cast-DMA, first on the gpsimd queue -- it
            # gates the whole AllGather chain.
            xnb = sb.tile([P, KN, BL], bf16)
            nc.gpsimd.dma_start(xnb[:], numT.rearrange("(k p) r -> p k r", p=P))
            # img weights next (needed when img MLP1 starts)
            wi1_sb = sb.tile([P, KI * P], bf16)
            nc.gpsimd.dma_start(wi1_sb.rearrange("p (k m) -> p k m", k=KI),
                                Wi1.rearrange("(k p) m -> p k m", p=P))
            wi2_sb = sb.tile([P, P], bf16)
            nc.gpsimd.dma_start(wi2_sb[:], Wi2)
            # img input, even k-chunks: SWDGE DMAs from t~0 -- the itl
            # chain (img DMA + 64-matmul MLP1 at cold PE clock + norm) is
            # co-critical with the collective chain, so the chunks cannot
            # afford to wait behind the AllGather trigger.
            xib = sb.tile([P, KI, BL], bf16)
            for k in range(0, KI, 2):
                nc.gpsimd.dma_start(xib[:, k, :], imgT[k * P:(k + 1) * P, :])

            # ---------------- constants / small loads (sync+DVE) --------
            ones_kb = sb.tile([P, 1], bf16)
            nc.vector.memset(ones_kb[:], 1.0)
            ones_kf = sb.tile([P, 1], f32)
            nc.vector.memset(ones_kf[:], 1.0)
            ones_1b = sb.tile([1, P], bf16)
            nc.vector.memset(ones_1b[:], 1.0)
            nc.sync.dma_start(ar_in[:1, b_total + 2:ARW], zpad[:1, :62])

            wn1_sb = sb.tile([P, KN * P], bf16)
            nc.sync.dma_start(wn1_sb.rearrange("p (k m) -> p k m", k=KN),
                              Wn1.rearrange("(k p) m -> p k m", p=P))
            wn2_sb = sb.tile([P, P], bf16)
            nc.sync.dma_start(wn2_sb[:], Wn2)
            bn1_sb = sb.tile([P, 1], f32)
            nc.sync.dma_start(bn1_sb[:], bn1)
            bn2_sb = sb.tile([P, 1], f32)
            nc.sync.dma_start(bn2_sb[:], bn2)
            lt_sb = sb.tile([1, 1], f32)
            nc.sync.dma_start(lt_sb[:], ltm)
            nhlt = sb.tile([1, 1], f32)        # -0.5 * log_temp
            nc.vector.tensor_scalar_mul(nhlt[:], lt_sb[:], -0.5)
            bi1_sb = sb.tile([P, 1], f32)
            nc.sync.dma_start(bi1_sb[:], bi1)
            bi2_sb = sb.tile([P, 1], f32)
            nc.sync.dma_start(bi2_sb[:], bi2)


            # ---------------- persistent SBUF ----------------
            h1n = sb.tile([P, BL], bf16)
            h1i = sb.tile([P, BL], bf16)
            zn = sb.tile([P, BL], bf16)
            zi = sb.tile([P, BL], bf16)
            ntl = sb.tile([P, BL], bf16)        # normalized num proj (local)
            itl = sb.tile([P, BL], bf16)        # normalized img proj (local)
            npf = sb.tile([P, b_total], bf16)   # gathered num proj (all cores)
            vsq = sb.tile([1, BL], f32)         # per-row |z|^2 staging
            inv_b = sb.tile([1, BL], bf16)      # per-row 1/(|z| sqrt(temp))
            rowacc = sb.tile([P, NRC * NCT], f32)
            csb = sb.tile([P, b_total // P], f32)
            dsum = sb.tile([1, 1], f32)         # running sum of diag
            nc.vector.memset(dsum[:], 0.0)

            def mlp2_norm(pp, h1, w2, b2, z, outp):
                """z = w2.T@h1 + b2 (transposed layout); outp = z * inv, with
                inv[i] = exp(-0.5*ln(|z_i|^2) - 0.5*log_temp).  The Ln/Exp
                pair is batched over the whole branch so the ACT table set
                switches only twice per branch instead of twice per row
                tile."""
                for rt in range(NRT):
                    sl = slice(rt * 512, (rt + 1) * 512)
                    pz = pp.tile([P, 512], f32, tag="zb", name="pz")
                    nc.tensor.matmul(pz[:], w2[:], h1[:, sl])
                    nc.scalar.activation(z[:, sl], pz[:], AF.Identity, bias=b2[:])
                    sq = st.tile([P, 512], bf16, tag="sq", name="sq")
                    nc.scalar.activation(sq[:], pz[:], AF.Square, bias=b2[:])
                    pv = pp.tile([P, 512], f32, tag="v", name="pv")
                    nc.tensor.matmul(pv[:1, :], ones_kb[:], sq[:])
                    nc.vector.tensor_copy(vsq[:1, sl], pv[:1, :])
                nc.scalar.activation(vsq[:], vsq[:], AF.Ln)
                nc.scalar.activation(inv_b[:], vsq[:], AF.Exp,
                                     bias=nhlt[:], scale=-0.5)
                for rt in range(NRT):
                    sl = slice(rt * 512, (rt + 1) * 512)
                    pb = pp.tile([P, 512], f32, tag="zb", name="pb")
                    nc.tensor.matmul(pb[:], ones_1b[:], inv_b[:1, sl])
                    nc.vector.tensor_mul(outp[:, sl], z[:, sl], pb[:])

            with tc.tile_pool(name="pp", bufs=2, space="PSUM") as pp:
                # ---------------- num branch + AllGather ----------------
                for rt in range(NRT):
                    sl = slice(rt * 512, (rt + 1) * 512)
                    ph = pp.tile([P, 512], f32, tag="h", name="ph", bufs=4)
                    for k in range(KN):
                        nc.tensor.matmul(ph[:], wn1_sb[:, k * P:(k + 1) * P],
                                         xnb[:, k, sl],
                                         start=(k == 0), stop=(k == KN - 1))
                    nc.scalar.activation(h1n[:, sl], ph[:], AF.Relu,
                                         bias=bn1_sb[:])
                mlp2_norm(pp, h1n, wn2_sb, bn2_sb, zn, ntl)

                # ag_in DMAs ride the scalar queue (HWDGE): the scalar
                # engine reaches them right after the num branch finishes,
                # and the sync queue is busy streaming img chunks.  The
                # gather is split: a small early slice (SL0 cols of every
                # core's block) unblocks the first column supertile ~15us
                # before the bulk slice lands.  npf is REPACKED (slice-a
                # blocks first, then slice-b blocks); the AllReduce adds
                # matching repacked positions, which are the same global
                # column on every core, and sum_j ln(colsum_j) is
                # order-independent, so nothing downstream unscrambles.
                nc.scalar.dma_start(ag_in_a[:], ntl[:, 0:SL0])
                nc.scalar.dma_start(ag_in_b[:], ntl[:, SL0:BL])
                # img odd k-chunks on the sync HWDGE queue, issued here
                # (after the num branch in program order) so the DMA paths
                # stay quiet while the collective bootstrap runs.
                for k in range(1, KI, 2):
                    nc.sync.dma_start(xib[:, k, :], imgT[k * P:(k + 1) * P, :])
                nc.gpsimd.collective_compute(
                    "AllGather", Alu.bypass, replica_groups=rg,
                    ins=[ag_in_a.opt()], outs=[ag_out_a.opt()])
                nc.gpsimd.collective_compute(
                    "AllGather", Alu.bypass, replica_groups=rg,
                    ins=[ag_in_b.opt()], outs=[ag_out_b.opt()])
                # unpack: slice-a blocks repack into npf[0 : 8*SL0],
                # slice-b blocks follow
                SLB = BL - SL0
                for c in range(n_cores):
                    nc.sync.dma_start(npf[:, c * SL0:(c + 1) * SL0],
                                      ag_out_a[c * P:(c + 1) * P, :])
                for c in range(n_cores):
                    base = n_cores * SL0 + c * SLB
                    nc.sync.dma_start(npf[:, base:base + SLB],
                                      ag_out_b[c * P:(c + 1) * P, :])

                # ---------------- img branch ----------------
                # contraction-outer MLP1: each k-chunk is consumed as it
                # arrives; the NRT row-tiles accumulate in parallel PSUM
                # banks across the whole k loop.
                ph_list = [pp.tile([P, 512], f32, tag="h", name=f"phi{rt}",
                                   bufs=4) for rt in range(NRT)]
                for k in range(KI):
                    for rt in range(NRT):
                        nc.tensor.matmul(
                            ph_list[rt][:], wi1_sb[:, k * P:(k + 1) * P],
                            xib[:, k, rt * 512:(rt + 1) * 512],
                            start=(k == 0), stop=(k == KI - 1))
                for rt in range(NRT):
                    nc.scalar.activation(h1i[:, rt * 512:(rt + 1) * 512],
                                         ph_list[rt][:], AF.Relu,
                                         bias=bi1_sb[:])
                mlp2_norm(pp, h1i, wi2_sb, bi2_sb, zi, itl)
                # diagonal: l_ii = sum_p itl[p,i] * ntl[p,i]; accumulate sum
                for rt in range(NRT):
                    sl = slice(rt * 512, (rt + 1) * 512)
                    prod = st.tile([P, 512], bf16, tag="sq", name="prod")
                    nc.vector.tensor_mul(prod[:], itl[:, sl], ntl[:, sl])
                    pd = pp.tile([P, 512], f32, tag="v", name="pd")
                    nc.tensor.matmul(pd[:1, :], ones_kb[:], prod[:])
                    dred = vs.tile([1, 1], f32, tag="dred", name="dred")
                    nc.vector.reduce_sum(dred[:], pd[:1, :], axis=AX.X)
                    nc.vector.tensor_add(dsum[:], dsum[:], dred[:])

            # ---------------- main pass ----------------
            # Per supertile ct (2048 cols): 16 row-chunk tiles. PE computes
            # logits into PSUM, ACT exponentiates (rowsums via accum_out),
            # DVE folds e-tiles into a bf16 column accumulator. The colsum
            # finalize (4 ones-matmuls) is deferred past the next ct's first
            # tile to keep it off the PE queue's critical path; colsums are
            # DMA'd to the AllReduce buffer straight from PSUM.
            with tc.tile_pool(name="pl", bufs=2, space="PSUM") as plp:
                pending = None          # (acc, ct) awaiting colsum finalize

                def finalize_colsum(acc, e_last, ct):
                    # the last row-chunk's e-tile is folded in via PSUM
                    # accumulation instead of a DVE add, so the finalize
                    # waits only on its Exp, not on the accumulator chain
                    pcs = plp.tile([P, CW], f32, tag="pl", name="pcs")
                    for h in range(NH):
                        sl = slice(h * 512, (h + 1) * 512)
                        nc.tensor.matmul(pcs[:1, sl], ones_kb[:], acc[:, sl],
                                         start=True, stop=False)
                        nc.tensor.matmul(pcs[:1, sl], ones_kb[:],
                                         e_last[:, sl],
                                         start=False, stop=True)
                    cst = vs.tile([1, CW], f32, tag="cst", name="cst", bufs=1)
                    nc.vector.tensor_copy(cst[:], pcs[:1, :])
                    nc.sync.dma_start(ar_in[:1, ct * CW:(ct + 1) * CW], cst[:])
                    if ct == NCT // 2 - 1:
                        # first half of colsums complete -> overlap AllReduce
                        nc.gpsimd.collective_compute(
                            "AllReduce", Alu.add, replica_groups=rg,
                            ins=[ar_in[:1, 0:ARH].opt()], outs=[ar_out_a.opt()])
                        nc.sync.dma_start(
                            csb[:, :ARH // P],
                            ar_out_a.rearrange("o (a b) -> (o a) b", a=P))

                for ct in range(NCT):
                    acc = vs.tile([P, CW], bf16, tag="acc", name="acc")
                    for rc in range(NRC):
                        plog = plp.tile([P, CW], f32, tag="pl", name="plog")
                        for h in range(NH):
                            nc.tensor.matmul(
                                plog[:, h * 512:(h + 1) * 512],
                                itl[:, rc * P:(rc + 1) * P],
                                npf[:, ct * CW + h * 512: ct * CW + (h + 1) * 512])
                        e = st.tile([P, CW], bf16, tag="e", name="e", bufs=6)
                        slot = rc * NCT + ct
                        nc.scalar.activation(e[:], plog[:], AF.Exp,
                                             accum_out=rowacc[:, slot:slot + 1])
                        if rc == 0:
                            nc.vector.tensor_copy(acc[:], e[:])
                        elif rc < NRC - 1:
                            nc.vector.tensor_add(acc[:], acc[:], e[:])
                        else:
                            e_last = e
                        if rc == 1 and pending is not None:
                            finalize_colsum(*pending)
                            pending = None
                    pending = (acc, e_last, ct)
                finalize_colsum(*pending)

                # ---- row direction partials ----
                # dummy Ln preloads the ln table set while the DVE reduce
                # runs, hiding the ~1.3us ACT table switch
                tln = sb.tile([1, 1], f32)
                nc.scalar.activation(tln[:], ones_kf[:1, :1], AF.Ln)
                rowsum = sb.tile([P, NRC], f32)
                nc.vector.reduce_sum(
                    rowsum[:],
                    rowacc.rearrange("p (rc ct) -> p rc ct", ct=NCT), axis=AX.X)
                lse_r = sb.tile([P, NRC], f32)
                nc.scalar.activation(lse_r[:], rowsum[:], AF.Ln)
                lsum = sb.tile([P, 1], f32)
                nc.vector.reduce_sum(lsum[:], lse_r[:], axis=AX.X)
                pR = plp.tile([P, CW], f32, tag="pl", name="pR")
                nc.tensor.matmul(pR[:1, :1], ones_kf[:], lsum[:])
                rd2 = sb.tile([1, 2], f32)
                nc.vector.tensor_sub(rd2[:1, 0:1], pR[:1, :1], dsum[:])
                nc.vector.tensor_copy(rd2[:1, 1:2], dsum[:])
                nc.sync.dma_start(ar_in[:1, b_total:b_total + 2], rd2[:])

                # ---- AllReduce (second half + scalars) ----
                nc.gpsimd.collective_compute(
                    "AllReduce", Alu.add, replica_groups=rg,
                    ins=[ar_in[:1, ARH:ARW].opt()], outs=[ar_out_b.opt()])

                # ---- final ----
                # sum_j log(colsum_j) is order-independent, so any colsum
                # layout works.
                HB2 = b_total - ARH
                nc.sync.dma_start(
                    csb[:, ARH // P:],
                    ar_out_b[:1, :HB2].rearrange("o (a b) -> (o a) b", a=P))
                sc2 = sb.tile([1, 2], f32)
                nc.sync.dma_start(sc2[:], ar_out_b[:1, HB2:HB2 + 2])
                lse_c = sb.tile([P, b_total // P], f32)
                nc.scalar.activation(lse_c[:], csb[:], AF.Ln)
                csum_p = sb.tile([P, 1], f32)
                nc.vector.reduce_sum(csum_p[:], lse_c[:], axis=AX.X)
                pC = plp.tile([P, CW], f32, tag="pl", name="pC")
                nc.tensor.matmul(pC[:1, :1], ones_kf[:], csum_p[:])
                t1 = sb.tile([1, 1], f32)
                nc.vector.tensor_add(t1[:], pC[:1, :1], sc2[:1, 0:1])
                t2 = sb.tile([1, 1], f32)
                nc.vector.tensor_sub(t2[:], t1[:], sc2[:1, 1:2])
                lsb = sb.tile([1, 1], f32)
                nc.vector.tensor_scalar_mul(lsb[:], t2[:], 1.0 / (2.0 * b_total))
                nc.sync.dma_start(loss, lsb[:])

    nc.compile()
    _NC_CACHE[key] = nc
    return nc


def _bf16(x):
    """Cast f32 -> bfloat16 (round-to-nearest-even) host-side."""
    try:
        import ml_dtypes
        return np.asarray(x, dtype=np.float32).astype(ml_dtypes.bfloat16)
    except ImportError:
        f = np.ascontiguousarray(np.asarray(x, dtype=np.float32))
        u = f.view(np.uint32)
        r = ((u >> 16) & 1) + 0x7FFF
        return ((u + r) >> 16).astype(np.uint16)


def shard_inputs(inputs, b_total=B, n_cores=N_CORES):
    BL = b_total // n_cores
    img = np.asarray(inputs["img_feat"], dtype=np.float32)
    num = np.asarray(inputs["num_feat"], dtype=np.float32)

    def col(name):
        return np.ascontiguousarray(
            np.asarray(inputs[name], dtype=np.float32).reshape(P, 1))

    lt = np.asarray(inputs["log_temp"], dtype=np.float32).reshape(1, 1)
    shared = {
        "Wi1": _bf16(inputs["Wi1"]), "Wi2": _bf16(inputs["Wi2"]),
        "Wn1": _bf16(inputs["Wn1"]), "Wn2": _bf16(inputs["Wn2"]),
        "bi1": col("bi1"), "bi2": col("bi2"),
        "bn1": col("bn1"), "bn2": col("bn2"),
        "log_temp": np.ascontiguousarray(lt),
    }
    maps = []
    for c in range(n_cores):
        m = dict(shared)
        m["imgT"] = np.ascontiguousarray(_bf16(img[c * BL:(c + 1) * BL]).T)
        m["numT"] = np.ascontiguousarray(_bf16(num[c * BL:(c + 1) * BL]).T)
        maps.append(m)
    return maps


def run(inputs, trace=False, **kw):
    """Run on hardware; returns (loss_scalar, BassKernelResults)."""
    try:
        from concourse.bass_utils import run_bass_kernel_spmd
    except ImportError:
        import sys
        sys.path.insert(0, "/opt/trn_rl_repo")
        from concourse.bass_utils import run_bass_kernel_spmd
    nc = build()
    res = run_bass_kernel_spmd(nc, shard_inputs(inputs),
                               core_ids=list(range(N_CORES)), trace=trace, **kw)
    val = np.asarray(res.results[0]["loss"], dtype=np.float32).reshape(())
    return val, res


def kernel(**inputs):
    val, _ = run(inputs)
    return val
